# revision 19
# baseline (speedup 1.0000x reference)
"""Trainium2 Bass kernel for nn_CrossAttention (B=32, TF=2048, TP=256,
FRAME=768, PHN=512, ATT=512), data-parallel over batch on 8 NeuronCores.

Math per batch element (matches the jax reference):
    q  = frame @ Wq + bq                 [TF, A]
    k  = phn @ Wk + bk                   [TP, A]
    energy = q @ k.T + (1-mask)*(-1000)  [TF, TP]   (returned)
    att = softmax(energy, -1)
    out = LN(concat[att @ k, q]) * gamma + beta     (returned)

Device-side decomposition (avoids materializing q.T):
    kT = contraction of Wk with phnT     [A, TP]
    W2 = Wq @ kT                         [F, TP]
    bias_row = bq @ kT + maskbias        [TP]
    energy = frame @ W2 + ones (x) bias_row   (rank-1 matmul adds row bias)

All matmul operands are bf16 (fp32 PSUM accumulation); every transpose
(frame, phn, att, Wq) runs on the DMA xbar (2-byte dtype), so the PE does
only matmuls. Inputs are cast fp32->bf16 inline by SWDGE DMA on load.
"""

import numpy as np

import concourse.bass as bass
import concourse.tile as tile
from concourse import mybir
from concourse.bass_utils import run_bass_kernel_spmd
from concourse.masks import make_identity
from concourse.vector_clock import ScopedClock, VectorClock

F32 = mybir.dt.float32
BF16 = mybir.dt.bfloat16
I32 = mybir.dt.int32
AF = mybir.ActivationFunctionType
AX = mybir.AxisListType
ALU = mybir.AluOpType

B, TF, TP = 32, 2048, 256
FD, PD, AD = 768, 512, 512
N_CORES = 8
BPC = B // N_CORES          # batch elements per core
NTC = TF // 512             # 512-row t-chunks per batch element
NF = FD // 128              # 6 f-tiles
NA = AD // 128              # 4 a-tiles
NP = TP // 128              # 2 p-tiles


def _patch_drain():
    """The packaged walrus rejects the TileContext tail Drain when it
    carries >1 sem wait; absorb the waits on single-wait SP nops first."""
    if getattr(tile.TileContext, "_drain_patched", False):
        return

    def _drain_and_barrier(self, tick_clock, wait_clock):
        vec = tick_clock.global_clock
        n = len(vec)
        for proc in range(n):
            tck = vec[proc]
            if tck <= 0:
                continue
            req = VectorClock([0] * n)
            req.require_at_least(proc, tck)
            nop = self.nc.sync.nop(nofuse=True, hint="drain_split_wait")
            wait_clock.add_sem_waits(nop.ins, ScopedClock({None: req}))
        self.nc.sync.drain()
        self.nc.all_engine_barrier()
        assert self.sems is not None
        popped = self.nc._tile_sem_poison_stack.pop()
        assert popped is self._sem_poison
        self.nc.clear_and_free_semaphores(list(self.sems.allocated().values()))
        self.nc.all_engine_barrier()

    tile.TileContext._drain_and_barrier = _drain_and_barrier
    tile.TileContext._drain_patched = True


_patch_drain()


def _split_excess_waits(nc, max_waits=1):
    """The packaged walrus rejects instructions carrying more than one sem
    wait; hoist extras onto same-engine NoOps placed just before them."""
    n_new = 0
    for f in nc.m.functions:
        for bb in f.blocks:
            insts = bb.instructions
            out = []
            changed = False
            for inst in insts:
                si = inst.sync_info
                if si is not None and len(si.on_wait) > max_waits:
                    waits = list(si.on_wait)
                    for w in waits[:-max_waits]:
                        nop = mybir.InstNoOp(
                            name=f"{inst.name}-wsplit{n_new}", ins=[], outs=[])
                        nop.engine = inst.engine
                        nop.sync_info = mybir.SyncInfo(
                            on_wait=[w], on_update=[])
                        out.append(nop)
                        n_new += 1
                    si.on_wait = waits[-max_waits:]
                    changed = True
                out.append(inst)
            if changed:
                bb.instructions = out
    return n_new


def build_program(sum_bq=0.0):
    nc = bass.Bass("TRN2", target_bir_lowering=False)

    frame_d = nc.dram_tensor("frame", [BPC, TF, FD], F32, kind="ExternalInput")
    phn_d = nc.dram_tensor("phn", [BPC, TP, PD], F32, kind="ExternalInput")
    mask_d = nc.dram_tensor("mask", [BPC, TP], I32, kind="ExternalInput")
    wq_d = nc.dram_tensor("wq", [FD, AD], F32, kind="ExternalInput")
    bq_d = nc.dram_tensor("bq", [AD], F32, kind="ExternalInput")
    wk_d = nc.dram_tensor("wk", [PD, AD], F32, kind="ExternalInput")
    bk_d = nc.dram_tensor("bk", [AD], F32, kind="ExternalInput")
    attout_d = nc.dram_tensor("attout", [BPC, TF, 2 * AD], F32,
                              kind="ExternalOutput")
    energy_d = nc.dram_tensor("energy", [BPC, TF, TP], F32,
                              kind="ExternalOutput")

    with (
        tile.TileContext(nc) as tc,
        tc.tile_pool(name="consts", bufs=1) as consts,
        tc.tile_pool(name="batchl", bufs=4) as batchl,   # k/W2/bias (live)
        tc.tile_pool(name="batcht", bufs=2) as batcht,   # phoneme temporaries
        tc.tile_pool(name="loadp", bufs=3) as loadp,     # bf16 frame loads
        tc.tile_pool(name="ftp", bufs=4) as ftp,         # frameT tiles
        tc.tile_pool(name="catp", bufs=3) as catp,
        tc.tile_pool(name="chunkp", bufs=3) as chunkp,
        tc.tile_pool(name="psA", bufs=2, space="PSUM") as psA,   # frameT
        tc.tile_pool(name="psB", bufs=2, space="PSUM") as psB,   # q / k
        tc.tile_pool(name="psC", bufs=2, space="PSUM") as psC,   # energy/W2
        tc.tile_pool(name="psD", bufs=1, space="PSUM") as psD,   # attT/phnT
        tc.tile_pool(name="psE", bufs=1, space="PSUM") as psE,   # attout/kT
    ):
        # ---- constants --------------------------------------------------
        eps_col = consts.tile([128, 1], F32, tag="eps_col")
        nc.vector.memset(eps_col, 1e-5)
        ones_col = consts.tile([1, 128], BF16, tag="ones_col")
        nc.vector.memset(ones_col, 1.0)
        magic = consts.tile([128, 4], I32, tag="magic")
        nc.vector.memset(magic, 0x5F3759DF)
        ident = consts.tile([128, 128], BF16, tag="ident")
        make_identity(nc, ident)

        # weights: HWDGE fp32 load (staged in a cat slot) + ACT cast
        wq_sb = consts.tile([128, NF, AD], BF16, tag="wq")  # [f%128, fj, a]
        w_st = catp.tile([128, 4, 2 * AD], F32, tag="cat")
        w_stv = w_st.rearrange("p a b -> p (a b)")
        nc.sync.dma_start(
            out=w_stv[:, :NF * AD].rearrange("p (n a) -> p n a", n=NF),
            in_=wq_d[:, :].rearrange("(n p) a -> p n a", p=128))
        nc.scalar.activation(out=wq_sb.rearrange("p n a -> p (n a)"),
                             in_=w_stv[:, :NF * AD], func=AF.Copy)
        wk_sb = consts.tile([128, NA, AD], BF16, tag="wk")
        w_st2 = catp.tile([128, 4, 2 * AD], F32, tag="cat")
        w_st2v = w_st2.rearrange("p a b -> p (a b)")
        nc.sync.dma_start(
            out=w_st2v[:, :NA * AD].rearrange("p (n a) -> p n a", n=NA),
            in_=wk_d[:, :].rearrange("(n p) a -> p n a", p=128))
        nc.scalar.activation(out=wk_sb.rearrange("p n a -> p (n a)"),
                             in_=w_st2v[:, :NA * AD], func=AF.Copy)
        bq_row = consts.tile([1, AD], BF16, tag="bq_row")
        nc.gpsimd.dma_start(out=bq_row, in_=bq_d[:].unsqueeze(0))
        bk_row = consts.tile([1, AD], BF16, tag="bk_row")
        nc.gpsimd.dma_start(out=bk_row, in_=bk_d[:].unsqueeze(0))
        bq_col = consts.tile([128, NA], BF16, tag="bq_col")  # [a%128, ai]
        nc.gpsimd.dma_start(out=bq_col,
                            in_=bq_d[:].rearrange("(a p) -> p a", p=128))
        bk_col = consts.tile([128, NA], F32, tag="bk_col")   # ACT bias only
        nc.sync.dma_start(out=bk_col,
                          in_=bk_d[:].rearrange("(a p) -> p a", p=128))

        # bq broadcast [128, 512] fp32 via one rank-1 matmul (for the
        # GpSimd q-bias add)
        bqb_ps = psB.tile([128, AD], F32, tag="q")
        nc.tensor.matmul(bqb_ps, ones_col, bq_row, start=True, stop=True)
        bq_bc = consts.tile([128, AD], F32, tag="bq_bc")
        nc.scalar.activation(out=bq_bc, in_=bqb_ps, func=AF.Copy)

        # WqT[(a%, fj, ai, f_j)] via PE transposes (one-time, 24 tiles)
        wqT_sb = consts.tile([128, NF, NA, 128], BF16, tag="wqT")
        for fj in range(NF):
            tp_ = psA.tile([128, 512], BF16, tag="ft")
            for ai in range(NA):
                nc.tensor.matmul(
                    tp_[:, ai * 128:(ai + 1) * 128],
                    wq_sb[:, fj, ai * 128:(ai + 1) * 128], ident,
                    is_transpose=True, start=(ai == 0), stop=(ai == NA - 1))
            nc.vector.tensor_copy(
                out=wqT_sb[:, fj, :, :].rearrange("p a j -> p (a j)"), in_=tp_)

        seq = [(b, ci) for b in range(BPC) for ci in range(NTC)]
        PF = 3
        ft_pending = {}

        def _load(idx):
            pb, pci = seq[idx]
            t0 = pci * 512
            # partition j holds rows t = 4j..4j+3 (contiguous 12KB fp32 per
            # partition); SWDGE casts to bf16 inline.
            fb = loadp.tile([128, 4 * FD], BF16, tag="frame")
            nc.gpsimd.dma_start(
                out=fb,
                in_=frame_d[pb, t0:t0 + 512, :].rearrange(
                    "(p s) f -> p (s f)", s=4))
            return fb

        def _transpose(idx, fb):
            fbv = fb.rearrange("p (s f) -> p s f", s=4)
            ftb = ftp.tile([128, NF, 512], BF16, tag="ft")
            for fj in range(NF):
                fp_ = psA.tile([128, 512], BF16, tag="ft")
                for s in range(4):
                    nc.tensor.matmul(
                        fp_[:, s * 128:(s + 1) * 128],
                        fbv[:, s, fj * 128:(fj + 1) * 128], ident,
                        is_transpose=True, start=(s == 0), stop=(s == 3))
                if fj % 2 == 0:
                    nc.scalar.activation(out=ftb[:, fj, :], in_=fp_,
                                         func=AF.Copy)
                else:
                    nc.vector.tensor_copy(out=ftb[:, fj, :], in_=fp_)
            ft_pending[idx] = ftb

        fb_pending = {}
        for _pf in range(PF):
            fb_pending[_pf] = _load(_pf)
        # transpose chunk 0 ahead of the phoneme stages so the PE has work
        # as soon as the first frame tile lands
        _transpose(0, fb_pending.pop(0))

        # ---- phoneme-side stages (all batch elements up front) ----------
        kT_all, k_all, w2_all, bias_all = [], [], [], []
        for b in range(BPC):
            phn_bf = batcht.tile([128, NP, PD], BF16, tag="phn")
            nc.gpsimd.dma_start(
                out=phn_bf,
                in_=phn_d[b, :, :].rearrange("(s p) f -> p s f", p=128))
            # phnT [f%128, fj, p] via PE transposes (8 tiles)
            phnT_sb = batcht.tile([128, NA, TP], BF16, tag="phnT")
            for fj in range(NA):
                pp_ = psD.tile([128, TP], BF16, tag="at")
                for pi in range(NP):
                    nc.tensor.matmul(
                        pp_[:, pi * 128:(pi + 1) * 128],
                        phn_bf[:, pi, fj * 128:(fj + 1) * 128], ident,
                        is_transpose=True, start=(pi == 0), stop=(pi == NP - 1))
                nc.vector.tensor_copy(out=phnT_sb[:, fj, :], in_=pp_)

            # kT[a, p] = sum_f Wk[f,a] phnT[f,p]; +bk along partitions
            kT_sb = batcht.tile([128, NA, TP], BF16, tag="kT")
            for ai in range(NA):
                kt = psE.tile([128, TP], F32, tag="o")
                for fj in range(NA):
                    nc.tensor.matmul(
                        kt, wk_sb[:, fj, ai * 128:(ai + 1) * 128],
                        phnT_sb[:, fj, :],
                        start=(fj == 0), stop=(fj == NA - 1))
                nc.scalar.activation(
                    out=kT_sb[:, ai, :], in_=kt, func=AF.Identity,
                    bias=bk_col[:, ai:ai + 1])

            # k[p, a] = sum_f phnT[f,p] Wk[f,a] + ones (x) bk
            k_sb = batchl.tile([128, NP, AD], BF16, tag="k")
            for pi in range(NP):
                kp = psB.tile([128, AD], F32, tag="q")
                for fj in range(NA):
                    nc.tensor.matmul(
                        kp, phnT_sb[:, fj, pi * 128:(pi + 1) * 128],
                        wk_sb[:, fj, :],
                        start=(fj == 0), stop=False)
                nc.tensor.matmul(kp, ones_col, bk_row, start=False, stop=True)
                nc.scalar.activation(out=k_sb[:, pi, :], in_=kp, func=AF.Copy)

            # W2[f, p] = sum_a Wq[f,a] kT[a,p]
            w2_sb = batchl.tile([128, NF, TP], BF16, tag="w2")
            for fj in range(NF):
                wp = psC.tile([128, TP], F32, tag="e")
                for ai in range(NA):
                    nc.tensor.matmul(
                        wp, wqT_sb[:, fj, ai, :], kT_sb[:, ai, :],
                        start=(ai == 0), stop=(ai == NA - 1))
                nc.scalar.activation(out=w2_sb[:, fj, :], in_=wp, func=AF.Copy)

            # bias_row = bq @ kT + (mask-1)*1000
            mask_i = batcht.tile([1, TP], I32, tag="mask_i")
            nc.sync.dma_start(out=mask_i, in_=mask_d[b, :].unsqueeze(0))
            bias_f = batcht.tile([1, TP], F32, tag="bias_f")
            nc.vector.tensor_copy(out=bias_f, in_=mask_i)  # int -> float
            nc.vector.tensor_scalar(
                out=bias_f, in0=bias_f, scalar1=1000.0, scalar2=-1000.0,
                op0=ALU.mult, op1=ALU.add)
            e0 = psE.tile([1, TP], F32, tag="o")
            for ai in range(NA):
                nc.tensor.matmul(
                    e0, bq_col[:, ai:ai + 1], kT_sb[:, ai, :],
                    start=(ai == 0), stop=(ai == NA - 1))
            bias_row = batcht.tile([1, TP], BF16, tag="bias_row")
            nc.vector.tensor_add(out=bias_row, in0=bias_f, in1=e0)
            bb_ps = psC.tile([128, TP], F32, tag="e")
            nc.tensor.matmul(bb_ps, ones_col, bias_row, start=True, stop=True)
            bias_bc = batchl.tile([128, TP], F32, tag="bias_bc")
            nc.scalar.activation(out=bias_bc, in_=bb_ps, func=AF.Copy)
            kT_all.append(kT_sb); k_all.append(k_sb)
            w2_all.append(w2_sb); bias_all.append(bias_bc)

        # ---- flat chunk pipeline ----------------------------------------
        for i, (b, ci) in enumerate(seq):
            k_sb, w2_sb, bias_bc = k_all[b], w2_all[b], bias_all[b]
            if i + PF < len(seq):
                fb_pending[i + PF] = _load(i + PF)
            if i in fb_pending:
                _transpose(i, fb_pending.pop(i))
            t0 = ci * 512
            ftb = ft_pending.pop(i)

            cat_sb = catp.tile([128, 4, 2 * AD], F32, tag="cat")
            acc_q = chunkp.tile([128, 4], F32, tag="acc_q")
            acc_o = chunkp.tile([128, 4], F32, tag="acc_o")
            acc_sq = chunkp.tile([128, 4], F32, tag="acc_sq")
            sq_scr = chunkp.tile([128, 2 * AD], F32, tag="sq_scr")
            energy_sb = chunkp.tile([128, 4, TP], F32, tag="energy")
            att_sb = chunkp.tile([128, 4, TP], BF16, tag="att")

            for ts in range(4):
                # q tile -> cat[:, ts, 512:1024]
                qp = psB.tile([128, AD], F32, tag="q")
                for fj in range(NF):
                    nc.tensor.matmul(
                        qp, ftb[:, fj, ts * 128:(ts + 1) * 128],
                        wq_sb[:, fj, :],
                        start=(fj == 0), stop=(fj == NF - 1))
                nc.scalar.activation(out=cat_sb[:, ts, AD:2 * AD], in_=qp,
                                     func=AF.Copy,
                                     accum_out=acc_q[:, ts:ts + 1])
                nc.gpsimd.tensor_add(
                    out=cat_sb[:, ts, AD:2 * AD],
                    in0=cat_sb[:, ts, AD:2 * AD], in1=bq_bc)

                # energy tile
                ep = psC.tile([128, TP], F32, tag="e")
                for fj in range(NF):
                    nc.tensor.matmul(
                        ep, ftb[:, fj, ts * 128:(ts + 1) * 128],
                        w2_sb[:, fj, :],
                        start=(fj == 0), stop=(fj == NF - 1))
                nc.vector.tensor_add(out=energy_sb[:, ts, :], in0=ep,
                                     in1=bias_bc)
                nc.scalar.activation(out=att_sb[:, ts, :],
                                     in_=energy_sb[:, ts, :], func=AF.Exp)

            nc.sync.dma_start(
                out=energy_d[b, t0:t0 + 512, :].rearrange(
                    "(p s) f -> p s f", s=4),
                in_=energy_sb)

            # softmax denominators
            s_sb = chunkp.tile([128, 4], F32, tag="s")
            nc.vector.reduce_sum(out=s_sb, in_=att_sb, axis=AX.X)
            r_sb = chunkp.tile([128, 4], F32, tag="r")
            nc.vector.reciprocal(out=r_sb, in_=s_sb)

            # attT [p%128, pi, t] via PE transposes (8 tiles)
            at_sb = chunkp.tile([128, NP, 512], BF16, tag="atT")
            for pi in range(NP):
                ap_ = psD.tile([128, 512], BF16, tag="at")
                for ts in range(4):
                    nc.tensor.matmul(
                        ap_[:, ts * 128:(ts + 1) * 128],
                        att_sb[:, ts, pi * 128:(pi + 1) * 128], ident,
                        is_transpose=True, start=(ts == 0), stop=(ts == 3))
                nc.vector.tensor_copy(out=at_sb[:, pi, :], in_=ap_)

            # att_out = (attU @ k) * r  -> cat[:, ts, 0:512]
            for ts in range(4):
                op_ = psE.tile([128, AD], F32, tag="o")
                for pi in range(NP):
                    nc.tensor.matmul(
                        op_, at_sb[:, pi, ts * 128:(ts + 1) * 128],
                        k_sb[:, pi, :],
                        start=(pi == 0), stop=(pi == NP - 1))
                nc.scalar.activation(out=cat_sb[:, ts, 0:AD], in_=op_,
                                     func=AF.Copy,
                                     scale=r_sb[:, ts:ts + 1],
                                     accum_out=acc_o[:, ts:ts + 1])

            # LayerNorm over 1024 (gamma/beta handled host-side).
            # First moments come free from the eviction accum_outs (plus the
            # baked-in sum(bq) for the GpSimd-added bias); second moment from
            # one ACT Square pass over the finished concat tile.
            for ts in range(4):
                nc.scalar.activation(out=sq_scr, in_=cat_sb[:, ts, :],
                                     func=AF.Square,
                                     accum_out=acc_sq[:, ts:ts + 1])
            mu = chunkp.tile([128, 4], F32, tag="mu")
            nc.vector.tensor_add(out=mu, in0=acc_q, in1=acc_o)
            nc.vector.tensor_scalar(
                out=mu, in0=mu, scalar1=sum_bq, scalar2=1.0 / (2 * AD),
                op0=ALU.add, op1=ALU.mult)
            x_t = chunkp.tile([128, 4], F32, tag="xt")
            nc.vector.tensor_mul(out=x_t, in0=mu, in1=mu)
            nc.vector.tensor_scalar(
                out=x_t, in0=x_t, scalar1=-1.0, scalar2=1e-5,
                op0=ALU.mult, op1=ALU.add)
            nc.vector.tensor_scalar(
                out=acc_sq, in0=acc_sq, scalar1=1.0 / (2 * AD), scalar2=None,
                op0=ALU.mult)
            nc.vector.tensor_add(out=x_t, in0=x_t, in1=acc_sq)
            rstd = chunkp.tile([128, 4], F32, tag="rstd")
            nc.vector.tensor_scalar(
                out=rstd.bitcast(I32), in0=x_t.bitcast(I32), scalar1=1,
                scalar2=None, op0=ALU.logical_shift_right)
            nc.vector.tensor_tensor(
                out=rstd.bitcast(I32), in0=magic, in1=rstd.bitcast(I32),
                op=ALU.subtract)
            h_t = chunkp.tile([128, 4], F32, tag="ht")
            for _ in range(2):
                nc.vector.tensor_mul(out=h_t, in0=rstd, in1=rstd)
                nc.vector.tensor_mul(out=h_t, in0=h_t, in1=x_t)
                nc.vector.tensor_scalar(
                    out=h_t, in0=h_t, scalar1=-0.5, scalar2=1.5,
                    op0=ALU.mult, op1=ALU.add)
                nc.vector.tensor_mul(out=rstd, in0=rstd, in1=h_t)
            nmr = chunkp.tile([128, 4], F32, tag="nmr")
            nc.vector.tensor_mul(out=nmr, in0=mu, in1=rstd)
            nc.vector.tensor_scalar_mul(out=nmr, in0=nmr, scalar1=-1.0)
            for ts in range(4):
                if ts < 2:
                    nc.vector.tensor_scalar(
                        out=cat_sb[:, ts, :], in0=cat_sb[:, ts, :],
                        scalar1=mu[:, ts:ts + 1], scalar2=rstd[:, ts:ts + 1],
                        op0=ALU.subtract, op1=ALU.mult)
                else:
                    nc.scalar.activation(
                        out=cat_sb[:, ts, :], in_=cat_sb[:, ts, :],
                        func=AF.Identity, bias=nmr[:, ts:ts + 1],
                        scale=rstd[:, ts:ts + 1])

            nc.sync.dma_start(
                out=attout_d[b, t0:t0 + 512, :].rearrange(
                    "(p s) f -> p s f", s=4),
                in_=cat_sb)

    _split_excess_waits(nc)
    return nc


_NC_CACHE = {}


def _get_program(sum_bq):
    key = round(float(sum_bq), 6)
    if key not in _NC_CACHE:
        _NC_CACHE[key] = build_program(sum_bq=float(sum_bq))
    return _NC_CACHE[key]


def make_in_maps(frame_hidden, phn_hidden, labels_att_mask, Wq, bq, Wk, bk):
    ins = []
    for c in range(N_CORES):
        s = slice(c * BPC, (c + 1) * BPC)
        ins.append({
            "frame": np.ascontiguousarray(frame_hidden[s], dtype=np.float32),
            "phn": np.ascontiguousarray(phn_hidden[s], dtype=np.float32),
            "mask": np.ascontiguousarray(labels_att_mask[s], dtype=np.int32),
            "wq": np.ascontiguousarray(Wq, dtype=np.float32),
            "bq": np.ascontiguousarray(bq, dtype=np.float32),
            "wk": np.ascontiguousarray(Wk, dtype=np.float32),
            "bk": np.ascontiguousarray(bk, dtype=np.float32),
        })
    return ins


def kernel(frame_hidden, phn_hidden, labels_att_mask, Wq, bq, Wk, bk,
           gamma, beta, _trace=False):
    nc = _get_program(np.float32(np.sum(np.asarray(bq, dtype=np.float32))))
    in_maps = make_in_maps(frame_hidden, phn_hidden, labels_att_mask,
                           Wq, bq, Wk, bk)
    res = run_bass_kernel_spmd(nc, in_maps, core_ids=list(range(N_CORES)),
                               trace=_trace)
    att_out = np.concatenate(
        [res.results[c]["attout"] for c in range(N_CORES)], 0)
    energy = np.concatenate(
        [res.results[c]["energy"] for c in range(N_CORES)], 0)
    gamma = np.asarray(gamma, dtype=np.float32)
    beta = np.asarray(beta, dtype=np.float32)
    if not (np.all(gamma == 1.0) and np.all(beta == 0.0)):
        att_out = att_out * gamma + beta
    if _trace:
        return (att_out, energy), res
    return (att_out, energy)


# revision 20
# speedup vs baseline: 1.4330x; 1.4330x over previous
"""Trainium2 Bass kernel for nn_CrossAttention (B=32, TF=2048, TP=256,
FRAME=768, PHN=512, ATT=512), data-parallel over batch on 8 NeuronCores.

Math per batch element (matches the jax reference):
    q  = frame @ Wq + bq                 [TF, A]
    k  = phn @ Wk + bk                   [TP, A]
    energy = q @ k.T + (1-mask)*(-1000)  [TF, TP]   (returned)
    att = softmax(energy, -1)
    out = LN(concat[att @ k, q]) * gamma + beta     (returned)

Device-side decomposition (avoids materializing q.T):
    kT = contraction of Wk with phnT     [A, TP]
    W2 = Wq @ kT                         [F, TP]
    bias_row = bq @ kT + maskbias        [TP]
    energy = frame @ W2 + ones (x) bias_row   (rank-1 matmul adds row bias)

All matmul operands are bf16 (fp32 PSUM accumulation); every transpose
(frame, phn, att, Wq) runs on the DMA xbar (2-byte dtype), so the PE does
only matmuls. Inputs are cast fp32->bf16 inline by SWDGE DMA on load.
"""

import numpy as np

import concourse.bass as bass
import concourse.tile as tile
from concourse import mybir
from concourse.bass_utils import run_bass_kernel_spmd
from concourse.masks import make_identity
from concourse.vector_clock import ScopedClock, VectorClock

F32 = mybir.dt.float32
BF16 = mybir.dt.bfloat16
I32 = mybir.dt.int32
AF = mybir.ActivationFunctionType
AX = mybir.AxisListType
ALU = mybir.AluOpType

B, TF, TP = 32, 2048, 256
FD, PD, AD = 768, 512, 512
N_CORES = 8
BPC = B // N_CORES          # batch elements per core
NTC = TF // 512             # 512-row t-chunks per batch element
NF = FD // 128              # 6 f-tiles
NA = AD // 128              # 4 a-tiles
NP = TP // 128              # 2 p-tiles


def _patch_drain():
    """The packaged walrus rejects the TileContext tail Drain when it
    carries >1 sem wait; absorb the waits on single-wait SP nops first."""
    if getattr(tile.TileContext, "_drain_patched", False):
        return

    def _drain_and_barrier(self, tick_clock, wait_clock):
        vec = tick_clock.global_clock
        n = len(vec)
        for proc in range(n):
            tck = vec[proc]
            if tck <= 0:
                continue
            req = VectorClock([0] * n)
            req.require_at_least(proc, tck)
            nop = self.nc.sync.nop(nofuse=True, hint="drain_split_wait")
            wait_clock.add_sem_waits(nop.ins, ScopedClock({None: req}))
        self.nc.sync.drain()
        self.nc.all_engine_barrier()
        assert self.sems is not None
        popped = self.nc._tile_sem_poison_stack.pop()
        assert popped is self._sem_poison
        self.nc.clear_and_free_semaphores(list(self.sems.allocated().values()))
        self.nc.all_engine_barrier()

    tile.TileContext._drain_and_barrier = _drain_and_barrier
    tile.TileContext._drain_patched = True


_patch_drain()


def _split_excess_waits(nc, max_waits=1):
    """The packaged walrus rejects instructions carrying more than one sem
    wait; hoist extras onto same-engine NoOps placed just before them."""
    n_new = 0
    for f in nc.m.functions:
        for bb in f.blocks:
            insts = bb.instructions
            out = []
            changed = False
            for inst in insts:
                si = inst.sync_info
                if si is not None and len(si.on_wait) > max_waits:
                    waits = list(si.on_wait)
                    for w in waits[:-max_waits]:
                        nop = mybir.InstNoOp(
                            name=f"{inst.name}-wsplit{n_new}", ins=[], outs=[])
                        nop.engine = inst.engine
                        nop.sync_info = mybir.SyncInfo(
                            on_wait=[w], on_update=[])
                        out.append(nop)
                        n_new += 1
                    si.on_wait = waits[-max_waits:]
                    changed = True
                out.append(inst)
            if changed:
                bb.instructions = out
    return n_new


def build_program():
    nc = bass.Bass("TRN2", target_bir_lowering=False)

    frame_d = nc.dram_tensor("frame", [BPC, TF, FD], F32, kind="ExternalInput")
    phn_d = nc.dram_tensor("phn", [BPC, TP, PD], F32, kind="ExternalInput")
    mask_d = nc.dram_tensor("mask", [BPC, TP], I32, kind="ExternalInput")
    wq_d = nc.dram_tensor("wq", [FD, AD], F32, kind="ExternalInput")
    bq_d = nc.dram_tensor("bq", [AD], F32, kind="ExternalInput")
    wk_d = nc.dram_tensor("wk", [PD, AD], F32, kind="ExternalInput")
    bk_d = nc.dram_tensor("bk", [AD], F32, kind="ExternalInput")
    attout_d = nc.dram_tensor("attout", [BPC, TF, 2 * AD], F32,
                              kind="ExternalOutput")
    energy_d = nc.dram_tensor("energy", [BPC, TF, TP], F32,
                              kind="ExternalOutput")

    with (
        tile.TileContext(nc) as tc,
        tc.tile_pool(name="consts", bufs=1) as consts,
        tc.tile_pool(name="batchl", bufs=4) as batchl,   # k/W2/bias (live)
        tc.tile_pool(name="batcht", bufs=2) as batcht,   # phoneme temporaries
        tc.tile_pool(name="loadp", bufs=3) as loadp,     # bf16 frame loads
        tc.tile_pool(name="ftp", bufs=4) as ftp,         # frameT tiles
        tc.tile_pool(name="catp", bufs=3) as catp,
        tc.tile_pool(name="chunkp", bufs=3) as chunkp,
        tc.tile_pool(name="psA", bufs=2, space="PSUM") as psA,   # frameT
        tc.tile_pool(name="psB", bufs=2, space="PSUM") as psB,   # q / k
        tc.tile_pool(name="psC", bufs=2, space="PSUM") as psC,   # energy/W2
        tc.tile_pool(name="psD", bufs=1, space="PSUM") as psD,   # attT/phnT
        tc.tile_pool(name="psE", bufs=1, space="PSUM") as psE,   # attout/kT
    ):
        # ---- constants --------------------------------------------------
        eps_col = consts.tile([128, 1], F32, tag="eps_col")
        nc.vector.memset(eps_col, 1e-5)
        ones_col = consts.tile([1, 128], BF16, tag="ones_col")
        nc.vector.memset(ones_col, 1.0)
        magic = consts.tile([128, 4], I32, tag="magic")
        nc.vector.memset(magic, 0x5F3759DF)
        ident = consts.tile([128, 128], BF16, tag="ident")
        make_identity(nc, ident)

        # weights: HWDGE fp32 load (staged in a cat slot) + ACT cast
        wq_sb = consts.tile([128, NF, AD], BF16, tag="wq")  # [f%128, fj, a]
        w_st = catp.tile([128, 4, 2 * AD], F32, tag="cat")
        w_stv = w_st.rearrange("p a b -> p (a b)")
        nc.sync.dma_start(
            out=w_stv[:, :NF * AD].rearrange("p (n a) -> p n a", n=NF),
            in_=wq_d[:, :].rearrange("(n p) a -> p n a", p=128))
        nc.scalar.activation(out=wq_sb.rearrange("p n a -> p (n a)"),
                             in_=w_stv[:, :NF * AD], func=AF.Copy)
        wk_sb = consts.tile([128, NA, AD], BF16, tag="wk")
        w_st2 = catp.tile([128, 4, 2 * AD], F32, tag="cat")
        w_st2v = w_st2.rearrange("p a b -> p (a b)")
        nc.sync.dma_start(
            out=w_st2v[:, :NA * AD].rearrange("p (n a) -> p n a", n=NA),
            in_=wk_d[:, :].rearrange("(n p) a -> p n a", p=128))
        nc.scalar.activation(out=wk_sb.rearrange("p n a -> p (n a)"),
                             in_=w_st2v[:, :NA * AD], func=AF.Copy)
        bq_row = consts.tile([1, AD], BF16, tag="bq_row")
        nc.gpsimd.dma_start(out=bq_row, in_=bq_d[:].unsqueeze(0))
        bk_row = consts.tile([1, AD], BF16, tag="bk_row")
        nc.gpsimd.dma_start(out=bk_row, in_=bk_d[:].unsqueeze(0))
        bq_col = consts.tile([128, NA], BF16, tag="bq_col")  # [a%128, ai]
        nc.gpsimd.dma_start(out=bq_col,
                            in_=bq_d[:].rearrange("(a p) -> p a", p=128))
        bk_col = consts.tile([128, NA], F32, tag="bk_col")   # ACT bias only
        nc.sync.dma_start(out=bk_col,
                          in_=bk_d[:].rearrange("(a p) -> p a", p=128))

        # WqT[(a%, fj, ai, f_j)] via PE transposes (one-time, 24 tiles)
        wqT_sb = consts.tile([128, NF, NA, 128], BF16, tag="wqT")
        for fj in range(NF):
            tp_ = psA.tile([128, 512], BF16, tag="ft")
            for ai in range(NA):
                nc.tensor.matmul(
                    tp_[:, ai * 128:(ai + 1) * 128],
                    wq_sb[:, fj, ai * 128:(ai + 1) * 128], ident,
                    is_transpose=True, start=(ai == 0), stop=(ai == NA - 1))
            nc.vector.tensor_copy(
                out=wqT_sb[:, fj, :, :].rearrange("p a j -> p (a j)"), in_=tp_)

        seq = [(b, ci) for b in range(BPC) for ci in range(NTC)]
        PF = 3
        ft_pending = {}

        def _load(idx):
            pb, pci = seq[idx]
            t0 = pci * 512
            # partition j holds rows t = 4j..4j+3 (contiguous 12KB fp32 per
            # partition); SWDGE casts to bf16 inline.
            fb = loadp.tile([128, 4 * FD], BF16, tag="frame")
            nc.gpsimd.dma_start(
                out=fb,
                in_=frame_d[pb, t0:t0 + 512, :].rearrange(
                    "(p s) f -> p (s f)", s=4))
            return fb

        def _transpose(idx, fb):
            fbv = fb.rearrange("p (s f) -> p s f", s=4)
            ftb = ftp.tile([128, NF, 512], BF16, tag="ft")
            for fj in range(NF):
                fp_ = psA.tile([128, 512], BF16, tag="ft")
                for s in range(4):
                    nc.tensor.matmul(
                        fp_[:, s * 128:(s + 1) * 128],
                        fbv[:, s, fj * 128:(fj + 1) * 128], ident,
                        is_transpose=True, start=(s == 0), stop=(s == 3))
                if fj % 2 == 0:
                    nc.scalar.activation(out=ftb[:, fj, :], in_=fp_,
                                         func=AF.Copy)
                else:
                    nc.vector.tensor_copy(out=ftb[:, fj, :], in_=fp_)
            ft_pending[idx] = ftb

        fb_pending = {}
        for _pf in range(PF):
            fb_pending[_pf] = _load(_pf)
        # transpose chunk 0 ahead of the phoneme stages so the PE has work
        # as soon as the first frame tile lands
        _transpose(0, fb_pending.pop(0))

        # ---- phoneme-side stages (all batch elements up front) ----------
        kT_all, k_all, w2_all, bias_all = [], [], [], []
        for b in range(BPC):
            phn_bf = batcht.tile([128, NP, PD], BF16, tag="phn")
            nc.gpsimd.dma_start(
                out=phn_bf,
                in_=phn_d[b, :, :].rearrange("(s p) f -> p s f", p=128))
            # phnT [f%128, fj, p] via PE transposes (8 tiles)
            phnT_sb = batcht.tile([128, NA, TP], BF16, tag="phnT")
            for fj in range(NA):
                pp_ = psD.tile([128, TP], BF16, tag="at")
                for pi in range(NP):
                    nc.tensor.matmul(
                        pp_[:, pi * 128:(pi + 1) * 128],
                        phn_bf[:, pi, fj * 128:(fj + 1) * 128], ident,
                        is_transpose=True, start=(pi == 0), stop=(pi == NP - 1))
                nc.vector.tensor_copy(out=phnT_sb[:, fj, :], in_=pp_)

            # kT[a, p] = sum_f Wk[f,a] phnT[f,p]; +bk along partitions
            kT_sb = batcht.tile([128, NA, TP], BF16, tag="kT")
            for ai in range(NA):
                kt = psE.tile([128, TP], F32, tag="o")
                for fj in range(NA):
                    nc.tensor.matmul(
                        kt, wk_sb[:, fj, ai * 128:(ai + 1) * 128],
                        phnT_sb[:, fj, :],
                        start=(fj == 0), stop=(fj == NA - 1))
                nc.scalar.activation(
                    out=kT_sb[:, ai, :], in_=kt, func=AF.Identity,
                    bias=bk_col[:, ai:ai + 1])

            # k[p, a] = sum_f phnT[f,p] Wk[f,a] + ones (x) bk
            k_sb = batchl.tile([128, NP, AD], BF16, tag="k")
            for pi in range(NP):
                kp = psB.tile([128, AD], F32, tag="q")
                for fj in range(NA):
                    nc.tensor.matmul(
                        kp, phnT_sb[:, fj, pi * 128:(pi + 1) * 128],
                        wk_sb[:, fj, :],
                        start=(fj == 0), stop=False)
                nc.tensor.matmul(kp, ones_col, bk_row, start=False, stop=True)
                nc.scalar.activation(out=k_sb[:, pi, :], in_=kp, func=AF.Copy)

            # W2[f, p] = sum_a Wq[f,a] kT[a,p]
            w2_sb = batchl.tile([128, NF, TP], BF16, tag="w2")
            for fj in range(NF):
                wp = psC.tile([128, TP], F32, tag="e")
                for ai in range(NA):
                    nc.tensor.matmul(
                        wp, wqT_sb[:, fj, ai, :], kT_sb[:, ai, :],
                        start=(ai == 0), stop=(ai == NA - 1))
                nc.scalar.activation(out=w2_sb[:, fj, :], in_=wp, func=AF.Copy)

            # bias_row = bq @ kT + (mask-1)*1000
            mask_i = batcht.tile([1, TP], I32, tag="mask_i")
            nc.sync.dma_start(out=mask_i, in_=mask_d[b, :].unsqueeze(0))
            bias_f = batcht.tile([1, TP], F32, tag="bias_f")
            nc.vector.tensor_copy(out=bias_f, in_=mask_i)  # int -> float
            nc.vector.tensor_scalar(
                out=bias_f, in0=bias_f, scalar1=1000.0, scalar2=-1000.0,
                op0=ALU.mult, op1=ALU.add)
            e0 = psE.tile([1, TP], F32, tag="o")
            for ai in range(NA):
                nc.tensor.matmul(
                    e0, bq_col[:, ai:ai + 1], kT_sb[:, ai, :],
                    start=(ai == 0), stop=(ai == NA - 1))
            bias_row = batcht.tile([1, TP], BF16, tag="bias_row")
            nc.vector.tensor_add(out=bias_row, in0=bias_f, in1=e0)
            bb_ps = psC.tile([128, TP], F32, tag="e")
            nc.tensor.matmul(bb_ps, ones_col, bias_row, start=True, stop=True)
            bias_bc = batchl.tile([128, TP], F32, tag="bias_bc")
            nc.scalar.activation(out=bias_bc, in_=bb_ps, func=AF.Copy)
            kT_all.append(kT_sb); k_all.append(k_sb)
            w2_all.append(w2_sb); bias_all.append(bias_bc)

        # ---- flat chunk pipeline ----------------------------------------
        for i, (b, ci) in enumerate(seq):
            k_sb, w2_sb, bias_bc = k_all[b], w2_all[b], bias_all[b]
            if i + PF < len(seq):
                fb_pending[i + PF] = _load(i + PF)
            if i in fb_pending:
                _transpose(i, fb_pending.pop(i))
            t0 = ci * 512
            ftb = ft_pending.pop(i)

            cat_sb = catp.tile([128, 4, 2 * AD], F32, tag="cat")
            energy_sb = chunkp.tile([128, 4, TP], F32, tag="energy")
            att_sb = chunkp.tile([128, 4, TP], BF16, tag="att")

            for ts in range(4):
                # q tile -> cat[:, ts, 512:1024]
                qp = psB.tile([128, AD], F32, tag="q")
                for fj in range(NF):
                    nc.tensor.matmul(
                        qp, ftb[:, fj, ts * 128:(ts + 1) * 128],
                        wq_sb[:, fj, :],
                        start=(fj == 0), stop=False)
                nc.tensor.matmul(qp, ones_col, bq_row, start=False, stop=True)
                nc.scalar.activation(out=cat_sb[:, ts, AD:2 * AD], in_=qp,
                                     func=AF.Copy)

                # energy tile
                ep = psC.tile([128, TP], F32, tag="e")
                for fj in range(NF):
                    nc.tensor.matmul(
                        ep, ftb[:, fj, ts * 128:(ts + 1) * 128],
                        w2_sb[:, fj, :],
                        start=(fj == 0), stop=(fj == NF - 1))
                nc.vector.tensor_add(out=energy_sb[:, ts, :], in0=ep,
                                     in1=bias_bc)
                nc.scalar.activation(out=att_sb[:, ts, :],
                                     in_=energy_sb[:, ts, :], func=AF.Exp)

            nc.sync.dma_start(
                out=energy_d[b, t0:t0 + 512, :].rearrange(
                    "(p s) f -> p s f", s=4),
                in_=energy_sb)

            # softmax denominators
            s_sb = chunkp.tile([128, 4], F32, tag="s")
            nc.vector.reduce_sum(out=s_sb, in_=att_sb, axis=AX.X)
            r_sb = chunkp.tile([128, 4], F32, tag="r")
            nc.vector.reciprocal(out=r_sb, in_=s_sb)

            # attT [p%128, pi, t] via PE transposes (8 tiles)
            at_sb = chunkp.tile([128, NP, 512], BF16, tag="atT")
            for pi in range(NP):
                ap_ = psD.tile([128, 512], BF16, tag="at")
                for ts in range(4):
                    nc.tensor.matmul(
                        ap_[:, ts * 128:(ts + 1) * 128],
                        att_sb[:, ts, pi * 128:(pi + 1) * 128], ident,
                        is_transpose=True, start=(ts == 0), stop=(ts == 3))
                nc.vector.tensor_copy(out=at_sb[:, pi, :], in_=ap_)

            # att_out = (attU @ k) * r  -> cat[:, ts, 0:512]
            for ts in range(4):
                op_ = psE.tile([128, AD], F32, tag="o")
                for pi in range(NP):
                    nc.tensor.matmul(
                        op_, at_sb[:, pi, ts * 128:(ts + 1) * 128],
                        k_sb[:, pi, :],
                        start=(pi == 0), stop=(pi == NP - 1))
                nc.scalar.activation(out=cat_sb[:, ts, 0:AD], in_=op_,
                                     func=AF.Copy,
                                     scale=r_sb[:, ts:ts + 1])

            # LayerNorm over 1024 (gamma/beta handled host-side)
            stats = chunkp.tile([128, 4, 2, 6], F32, tag="stats")
            mv = chunkp.tile([128, 4, 2], F32, tag="mv")
            for ts in range(4):
                for g in range(2):
                    nc.vector.bn_stats(
                        out=stats[:, ts, g, :],
                        in_=cat_sb[:, ts, g * 512:(g + 1) * 512])
                nc.vector.bn_aggr(out=mv[:, ts, :], in_=stats[:, ts, :, :])
            # rstd = rsqrt(var + eps) via magic-constant + 2 Newton steps
            x_t = chunkp.tile([128, 4], F32, tag="xt")
            nc.vector.tensor_scalar(
                out=x_t, in0=mv[:, :, 1], scalar1=1e-5, scalar2=None,
                op0=ALU.add)
            rstd = chunkp.tile([128, 4], F32, tag="rstd")
            nc.vector.tensor_scalar(
                out=rstd.bitcast(I32), in0=x_t.bitcast(I32), scalar1=1,
                scalar2=None, op0=ALU.logical_shift_right)
            nc.vector.tensor_tensor(
                out=rstd.bitcast(I32), in0=magic, in1=rstd.bitcast(I32),
                op=ALU.subtract)
            h_t = chunkp.tile([128, 4], F32, tag="ht")
            for _ in range(2):
                nc.vector.tensor_mul(out=h_t, in0=rstd, in1=rstd)
                nc.vector.tensor_mul(out=h_t, in0=h_t, in1=x_t)
                nc.vector.tensor_scalar(
                    out=h_t, in0=h_t, scalar1=-0.5, scalar2=1.5,
                    op0=ALU.mult, op1=ALU.add)
                nc.vector.tensor_mul(out=rstd, in0=rstd, in1=h_t)
            nmr = chunkp.tile([128, 4], F32, tag="nmr")
            nc.vector.tensor_mul(out=nmr, in0=mv[:, :, 0], in1=rstd)
            nc.vector.tensor_scalar_mul(out=nmr, in0=nmr, scalar1=-1.0)
            for ts in range(4):
                if ts < 2:
                    nc.vector.tensor_scalar(
                        out=cat_sb[:, ts, :], in0=cat_sb[:, ts, :],
                        scalar1=mv[:, ts, 0:1], scalar2=rstd[:, ts:ts + 1],
                        op0=ALU.subtract, op1=ALU.mult)
                else:
                    nc.scalar.activation(
                        out=cat_sb[:, ts, :], in_=cat_sb[:, ts, :],
                        func=AF.Identity, bias=nmr[:, ts:ts + 1],
                        scale=rstd[:, ts:ts + 1])

            nc.sync.dma_start(
                out=attout_d[b, t0:t0 + 512, :].rearrange(
                    "(p s) f -> p s f", s=4),
                in_=cat_sb)

    _split_excess_waits(nc)
    return nc


_NC_CACHE = []


def _get_program():
    if not _NC_CACHE:
        _NC_CACHE.append(build_program())
    return _NC_CACHE[0]


def make_in_maps(frame_hidden, phn_hidden, labels_att_mask, Wq, bq, Wk, bk):
    ins = []
    for c in range(N_CORES):
        s = slice(c * BPC, (c + 1) * BPC)
        ins.append({
            "frame": np.ascontiguousarray(frame_hidden[s], dtype=np.float32),
            "phn": np.ascontiguousarray(phn_hidden[s], dtype=np.float32),
            "mask": np.ascontiguousarray(labels_att_mask[s], dtype=np.int32),
            "wq": np.ascontiguousarray(Wq, dtype=np.float32),
            "bq": np.ascontiguousarray(bq, dtype=np.float32),
            "wk": np.ascontiguousarray(Wk, dtype=np.float32),
            "bk": np.ascontiguousarray(bk, dtype=np.float32),
        })
    return ins


def kernel(frame_hidden, phn_hidden, labels_att_mask, Wq, bq, Wk, bk,
           gamma, beta, _trace=False):
    nc = _get_program()
    in_maps = make_in_maps(frame_hidden, phn_hidden, labels_att_mask,
                           Wq, bq, Wk, bk)
    res = run_bass_kernel_spmd(nc, in_maps, core_ids=list(range(N_CORES)),
                               trace=_trace)
    att_out = np.concatenate(
        [res.results[c]["attout"] for c in range(N_CORES)], 0)
    energy = np.concatenate(
        [res.results[c]["energy"] for c in range(N_CORES)], 0)
    gamma = np.asarray(gamma, dtype=np.float32)
    beta = np.asarray(beta, dtype=np.float32)
    if not (np.all(gamma == 1.0) and np.all(beta == 0.0)):
        att_out = att_out * gamma + beta
    if _trace:
        return (att_out, energy), res
    return (att_out, energy)
